# revision 22
# baseline (speedup 1.0000x reference)
"""ApproxRepSet kernel for 8 TRN2 NeuronCores.

reference:
  t = relu(X @ Wc)            # [B, P, H*E], k = e*H + h
  t = max over e              # [B, P, H]
  t = sum over p              # [B, H]
  t = relu(t @ w1 + b1); t = relu(t @ w2 + b2); out = t @ w3 + b3

Sharding: data-parallel over batch, 16 batches per core. Weights replicated.

Per-core layout tricks (all host-side, zero on-device transposes):
  - X shard [16*1024, 64] is packed as A[128, 8192]: partition 64*(r%2)+d,
    free r//2.  A 256-row block i lives at free cols [128i, 128i+128): even
    rows on partitions 0:64, odd rows on partitions 64:128.  Each half is
    directly the matmul stationary lhsT [K=64, M=128]; the two halves run
    concurrently via PE row tiling (tile_position (0,0) and (64,0)).
  - X/Wc cast to bf16 on host: halves DMA bytes and avoids the fp32
    double-pass through the PE (fp32 matmul lowers to 2 instructions).
  - Wc columns reordered k' = h*16 + e, so max over e is an innermost
    free-dim window reduce.  Wc stacked twice on partitions for row tiling.
  - The max-over-e pooling is the throughput wall (every Y element must
    stream through a 128-lane engine).  Blocks are split across three
    pipelines so DVE, ACT and GPSIMD all pool in parallel:
      path a: DVE tensor_reduce(max) straight from PSUM     (~1.17us/blk DVE)
      path b: ACT relu-cast PSUM->SBUF bf16, DVE TT-max tree (~1.15 ACT + ~0.78 DVE)
      path e: ACT relu-copy PSUM->SBUF f32, GPSIMD TT-max tree (~1.15 ACT + ~2 GPS)
  - Sum over p: ones-vector matmuls (lhsT = pooled tile [128, 32],
    rhs = ones [128,1]) accumulating S^T [32, 16] in PSUM.
  - MLP stays transposed end-to-end: h1^T = w1^T @ S^T etc., so w1/w2/w3 are
    used in natural layout as lhsT.
"""

import sys

import numpy as np

sys.path.insert(0, "/opt/trn_rl_repo")

import ml_dtypes
import concourse.bass as bass
import concourse.mybir as mybir
import concourse.tile as tile
from concourse import bacc
from concourse.bass_utils import run_bass_kernel_spmd

B, P, D = 128, 1024, 64
H, E = 32, 16
HE = H * E  # 512
NOUT = 10
NCORES = 8
BPC = B // NCORES  # 16 batches per core
R = BPC * P  # 16384 rows per core
NBLK = R // 256  # 64 blocks of 256 rows
FCHUNK = 2048  # free-dim cols per DMA chunk (= 16 blocks)

FP32 = mybir.dt.float32
BF16 = mybir.dt.bfloat16
AX = mybir.AxisListType
ALU = mybir.AluOpType
ACT_F = mybir.ActivationFunctionType

# pooling path per 2-block pair: a = DVE-direct reduce, b = ACT relu-cast + one
# batched DVE TT-max tree over both blocks
# (GPSIMD TensorTensor fails walrus codegen on this toolchain, so no GPSIMD path)
PAIR_PATTERN = ["a", "b", "b", "b", "b", "a", "b", "b"]

_cache = {}


def _tree_max4(nc, pool, src, dst_slice):
    """4-level DVE TT-max tree over innermost e=16 of src [128, 4, H, 16] bf16
    (two blocks), writing [128, 4, H] into dst_slice."""
    t1 = pool.tile([128, 4, H, 8], BF16, tag="t1", name="t1")
    nc.vector.tensor_tensor(t1[:], src[:, :, :, 0:8], src[:, :, :, 8:16], op=ALU.max)
    t2 = pool.tile([128, 4, H, 4], BF16, tag="t2", name="t2")
    nc.vector.tensor_tensor(t2[:], t1[:, :, :, 0:4], t1[:, :, :, 4:8], op=ALU.max)
    t3 = pool.tile([128, 4, H, 2], BF16, tag="t3", name="t3")
    nc.vector.tensor_tensor(t3[:], t2[:, :, :, 0:2], t2[:, :, :, 2:4], op=ALU.max)
    nc.vector.tensor_tensor(dst_slice, t3[:, :, :, 0], t3[:, :, :, 1], op=ALU.max)


def _build_nc():
    nc = bacc.Bacc(
        "TRN2", target_bir_lowering=False, debug=False, num_devices=NCORES
    )

    xa = nc.declare_dram_parameter("xa", [128, R // 2], BF16, isOutput=False)
    wc = nc.declare_dram_parameter("wc", [128, HE], BF16, isOutput=False)
    # packed MLP weights [64, 141] f32: w1 rows 0:32 cols 0:64, w2 cols 64:128,
    # w3 cols 128:138, b1 col 138, b2 col 139, b3 col 140 (rows 0:10)
    wmlp = nc.declare_dram_parameter("wmlp", [64, 141], FP32, isOutput=False)
    out = nc.declare_dram_parameter("out", [NOUT, BPC], FP32, isOutput=True)

    with tile.TileContext(nc) as tc:
        with (
            tc.tile_pool(name="const", bufs=1) as const_pool,
            tc.tile_pool(name="xa", bufs=2) as xa_pool,
            tc.tile_pool(name="mb", bufs=2) as mb_pool,
            tc.tile_pool(name="yb", bufs=3) as yb_pool,
            tc.tile_pool(name="tree", bufs=2) as tree_pool,
            tc.tile_pool(name="mlp", bufs=1) as mlp_pool,
            tc.tile_pool(name="ypsum", bufs=1, space=bass.MemorySpace.PSUM) as ypsum_pool,
            tc.tile_pool(name="spsum", bufs=1, space=bass.MemorySpace.PSUM) as spsum_pool,
        ):
            # --- persistent tiles; first xa chunk + wc first (critical path) ---
            xa_tiles = []
            for c in range(4):
                t = xa_pool.tile([128, FCHUNK], BF16, tag="xa", name="xa_sb")
                xa_tiles.append(t)
            for piece in range(4):
                lo, hi = piece * 512, (piece + 1) * 512
                nc.sync.dma_start(xa_tiles[0][:, lo:hi], xa[:, lo:hi])
            wc_sb = const_pool.tile([128, HE], BF16)
            nc.sync.dma_start(wc_sb[:], wc[:])
            ones_sb = const_pool.tile([128, 1], BF16)
            nc.vector.memset(ones_sb[:], 1.0)

            # one PSUM bank shared by the S^T accumulator and the MLP matmuls
            sm_psum = spsum_pool.tile([64, 512], FP32)
            s_psum = sm_psum[0:H, 0:BPC]  # S^T accumulator
            # 6 PSUM banks as one tensor; block i uses region i%3 ([., 1024])
            y_big = ypsum_pool.tile([128, 3 * 2 * HE], FP32)

            # Software-pipelined main loop over 32 pairs (2 blocks each).
            # Trees lag one pair behind their ACT casts so DVE never stalls
            # on a cast in flight; batch relu + p-sum fire once both pairs
            # of a batch have their pooled outputs in mb.
            mbs = [None] * (BPC)  # mb tile per batch
            pending_tree = None  # (ybf2, dst)
            pending_batch = []  # batches whose pooling is fully emitted

            def flush_tree():
                nonlocal pending_tree
                if pending_tree is not None:
                    ybf2, dst = pending_tree
                    _tree_max4(nc, tree_pool, ybf2, dst)
                    pending_tree = None

            def flush_batches():
                while pending_batch:
                    bi, mb_t = pending_batch.pop(0)
                    mbf = mb_t[:].rearrange("p a b -> p (a b)")
                    nc.vector.tensor_scalar_max(mbf, mbf, 0.0)
                    for j in range(8):
                        nc.tensor.matmul(
                            s_psum[:, bi : bi + 1],
                            mb_t[:, j, :],
                            ones_sb[:],
                            start=(j == 0),
                            stop=(j == 7),
                        )

            for pair in range(NBLK // 2):
                i = 2 * pair
                if i % (FCHUNK // 128) == 0 and i > 0:
                    c = i // (FCHUNK // 128)
                    nc.sync.dma_start(
                        xa_tiles[c][:], xa[:, c * FCHUNK : (c + 1) * FCHUNK]
                    )
                b_idx = i // 4
                slot = i % 4
                if slot == 0:
                    mbs[b_idx] = mb_pool.tile([128, 8, H], BF16, tag="mb", name="mb")
                mb = mbs[b_idx]

                path = PAIR_PATTERN[pair % len(PAIR_PATTERN)]
                ybf2 = None
                if path == "b":
                    ybf2 = yb_pool.tile([128, 4, H, E], BF16, tag="ybf", name="ybf")
                r0 = i % 3  # psum region of first block; second is (i+1)%3
                contiguous = r0 < 2
                for q in range(2):  # the two blocks of the pair
                    blk = i + q
                    xa_sb = xa_tiles[blk // (FCHUNK // 128)]
                    f0 = (blk % (FCHUNK // 128)) * 128
                    g0 = ((blk % 3) * 2) * HE
                    nc.tensor.matmul(
                        y_big[:, g0 : g0 + HE],
                        xa_sb[0:64, f0 : f0 + 128],
                        wc_sb[0:64, :],
                        start=True,
                        stop=True,
                    )
                    nc.tensor.matmul(
                        y_big[:, g0 + HE : g0 + 2 * HE],
                        xa_sb[64:128, f0 : f0 + 128],
                        wc_sb[64:128, :],
                        start=True,
                        stop=True,
                    )
                    sl = slot + q
                    if contiguous and q == 0:
                        continue  # evacuated together with block q=1
                    if contiguous:
                        # one op covers both blocks (regions r0, r0+1)
                        src = y_big[:, 2 * r0 * HE : (2 * r0 + 4) * HE]
                        if path == "a":
                            nc.vector.tensor_reduce(
                                mb[:, 2 * slot : 2 * slot + 4, :],
                                src.rearrange("p (t h e) -> p t h e", t=4, h=H, e=E),
                                axis=AX.X,
                                op=ALU.max,
                            )
                        else:
                            nc.scalar.activation(
                                ybf2[:].rearrange("p a b c -> p (a b c)"),
                                src,
                                ACT_F.Relu,
                            )
                    else:
                        src = y_big[:, g0 : g0 + 2 * HE]
                        if path == "a":
                            nc.vector.tensor_reduce(
                                mb[:, 2 * sl : 2 * sl + 2, :],
                                src.rearrange("p (t h e) -> p t h e", t=2, h=H, e=E),
                                axis=AX.X,
                                op=ALU.max,
                            )
                        else:
                            nc.scalar.activation(
                                ybf2[:, 2 * q : 2 * q + 2].rearrange(
                                    "p a b c -> p (a b c)"
                                ),
                                src,
                                ACT_F.Relu,
                            )
                if path == "b":
                    pending_tree = (ybf2, mb[:, 2 * slot : 2 * slot + 4, :])
                flush_tree()
                if slot + 1 == 3:
                    pending_batch.append((b_idx, mb))
                    flush_batches()
            flush_tree()
            flush_batches()

            # --- MLP tail (all transposed); weights arrive in one late DMA ---
            wmlp_sb = const_pool.tile([64, 141], FP32)
            nc.sync.dma_start(wmlp_sb[:], wmlp[:])
            w1_sb = wmlp_sb[0:H, 0:64]
            w2_sb = wmlp_sb[0:64, 64:128]
            w3_sb = wmlp_sb[0:64, 128 : 128 + NOUT]
            b1_sb = wmlp_sb[0:64, 138:139]
            b2_sb = wmlp_sb[0:64, 139:140]
            b3_sb = wmlp_sb[0:NOUT, 140:141]

            s_sb = mlp_pool.tile([H, BPC], FP32)
            nc.vector.tensor_copy(s_sb[:], s_psum[:])

            h1_ps = sm_psum[0:64, 64:80]
            nc.tensor.matmul(h1_ps, w1_sb, s_sb[:], start=True, stop=True)
            h1_sb = mlp_pool.tile([64, BPC], FP32)
            nc.scalar.activation(h1_sb[:], h1_ps, ACT_F.Relu, bias=b1_sb)

            h2_ps = sm_psum[0:64, 96:112]
            nc.tensor.matmul(h2_ps, w2_sb, h1_sb[:], start=True, stop=True)
            h2_sb = mlp_pool.tile([64, BPC], FP32)
            nc.scalar.activation(h2_sb[:], h2_ps, ACT_F.Relu, bias=b2_sb)

            o_ps = sm_psum[0:NOUT, 128:144]
            nc.tensor.matmul(o_ps, w3_sb, h2_sb[:], start=True, stop=True)
            o_sb = mlp_pool.tile([NOUT, BPC], FP32)
            nc.scalar.activation(o_sb[:], o_ps, ACT_F.Identity, bias=b3_sb)

            nc.sync.dma_start(out[:], o_sb[:])

    nc.compile()
    return nc


def _prep_shared(Wc, w1, b1, w2, b2, w3, b3):
    # reorder Wc columns: k = e*H + h  ->  k' = h*E + e
    Wc = np.asarray(Wc, dtype=np.float32)
    wc_r = np.ascontiguousarray(
        Wc.reshape(D, E, H).transpose(0, 2, 1).reshape(D, HE)
    )
    wc_stack = np.ascontiguousarray(
        np.concatenate([wc_r, wc_r], axis=0).astype(ml_dtypes.bfloat16)
    )
    wmlp = np.zeros((64, 141), np.float32)
    wmlp[0:H, 0:64] = np.asarray(w1, np.float32)
    wmlp[0:64, 64:128] = np.asarray(w2, np.float32)
    wmlp[0:64, 128 : 128 + NOUT] = np.asarray(w3, np.float32)
    wmlp[0:64, 138] = np.asarray(b1, np.float32)
    wmlp[0:64, 139] = np.asarray(b2, np.float32)
    wmlp[0:NOUT, 140] = np.asarray(b3, np.float32)
    return dict(wc=wc_stack, wmlp=wmlp)


def _pack_x(Xc):
    # Xc [BPC, P, D] -> A [128, R//2]: A[64*(r%2)+d, r//2] = Xc_flat[r, d]
    Xf = np.asarray(Xc, np.float32).reshape(R, D)
    A = Xf.reshape(R // 2, 2, D).transpose(1, 2, 0).reshape(128, R // 2)
    return np.ascontiguousarray(A.astype(ml_dtypes.bfloat16))


def run(X, Wc, w1, b1, w2, b2, w3, b3, trace=False):
    if "nc" not in _cache:
        _cache["nc"] = _build_nc()
    nc = _cache["nc"]

    shared = _prep_shared(Wc, w1, b1, w2, b2, w3, b3)
    in_maps = []
    for c in range(NCORES):
        m = dict(shared)
        m["xa"] = _pack_x(X[c * BPC : (c + 1) * BPC])
        in_maps.append(m)

    res = run_bass_kernel_spmd(
        nc, in_maps, core_ids=list(range(NCORES)), trace=trace
    )
    outs = [np.asarray(r["out"]).T for r in res.results]  # each [BPC, NOUT]
    full = np.concatenate(outs, axis=0).astype(np.float32)
    return full, res


def kernel(X, Wc, w1, b1, w2, b2, w3, b3):
    full, _ = run(X, Wc, w1, b1, w2, b2, w3, b3, trace=False)
    return full


# revision 24
# speedup vs baseline: 1.6522x; 1.6522x over previous
"""ApproxRepSet kernel for 8 TRN2 NeuronCores.

reference:
  t = relu(X @ Wc)            # [B, P, H*E], k = e*H + h
  t = max over e              # [B, P, H]
  t = sum over p              # [B, H]
  t = relu(t @ w1 + b1); t = relu(t @ w2 + b2); out = t @ w3 + b3

Sharding: data-parallel over batch, 16 batches per core. Weights replicated.

Per-core layout tricks (all host-side, zero on-device transposes):
  - X shard [16*1024, 64] is packed as A[128, 8192]: partition 64*(r%2)+d,
    free r//2.  A 256-row block i lives at free cols [128i, 128i+128): even
    rows on partitions 0:64, odd rows on partitions 64:128.  Each half is
    directly the matmul stationary lhsT [K=64, M=128]; the two halves run
    concurrently via PE row tiling (tile_position (0,0) and (64,0)).
  - X/Wc cast to bf16 on host: halves DMA bytes and avoids the fp32
    double-pass through the PE (fp32 matmul lowers to 2 instructions).
  - Wc columns reordered k' = h*16 + e, so max over e is an innermost
    free-dim window reduce.  Wc stacked twice on partitions for row tiling.
  - The max-over-e pooling is the throughput wall (every Y element must be
    read out of PSUM by DVE or ACT at ~1 elem/lane/cycle; DMA and GPSIMD
    have no PSUM route, and GPSIMD TensorTensor fails walrus codegen).
    Block pairs are split across two pipelines so DVE and ACT pool in
    parallel:
      path a: DVE tensor_reduce(max) straight from PSUM    (~1.17us/blk DVE)
      path b: ACT relu-cast PSUM->SBUF bf16, then one batched DVE TT-max
              tree per pair                        (~1.07 ACT + ~0.67 DVE /blk)
  - Sum over p: ones-vector matmuls (lhsT = pooled tile [128, 32],
    rhs = ones [128,1]) accumulating S^T [32, 16] in PSUM.
  - MLP stays transposed end-to-end: h1^T = w1^T @ S^T etc., so w1/w2/w3 are
    used in natural layout as lhsT.
"""

import sys

import numpy as np

sys.path.insert(0, "/opt/trn_rl_repo")

import ml_dtypes
import concourse.bass as bass
import concourse.mybir as mybir
import concourse.tile as tile
from concourse import bacc
from concourse.bass_utils import run_bass_kernel_spmd

B, P, D = 128, 1024, 64
H, E = 32, 16
HE = H * E  # 512
NOUT = 10
NCORES = 8
BPC = B // NCORES  # 16 batches per core
R = BPC * P  # 16384 rows per core
NBLK = R // 256  # 64 blocks of 256 rows
FCHUNK = 2048  # free-dim cols per DMA chunk (= 16 blocks)

FP32 = mybir.dt.float32
BF16 = mybir.dt.bfloat16
AX = mybir.AxisListType
ALU = mybir.AluOpType
ACT_F = mybir.ActivationFunctionType

# pooling path per 2-block pair: a = DVE-direct reduce, b = ACT relu-cast + one
# batched DVE TT-max tree over both blocks
# (GPSIMD TensorTensor fails walrus codegen on this toolchain, so no GPSIMD path)
PAIR_PATTERN = ["a", "b", "b", "b", "b", "a", "b", "b"]

_cache = {}


def _tree_max4(nc, pool, src, dst_slice):
    """4-level DVE TT-max tree over innermost e=16 of src [128, 4, H, 16] bf16
    (two blocks), writing [128, 4, H] into dst_slice."""
    t1 = pool.tile([128, 4, H, 8], BF16, tag="t1", name="t1")
    nc.vector.tensor_tensor(t1[:], src[:, :, :, 0:8], src[:, :, :, 8:16], op=ALU.max)
    t2 = pool.tile([128, 4, H, 4], BF16, tag="t2", name="t2")
    nc.vector.tensor_tensor(t2[:], t1[:, :, :, 0:4], t1[:, :, :, 4:8], op=ALU.max)
    t3 = pool.tile([128, 4, H, 2], BF16, tag="t3", name="t3")
    nc.vector.tensor_tensor(t3[:], t2[:, :, :, 0:2], t2[:, :, :, 2:4], op=ALU.max)
    nc.vector.tensor_tensor(dst_slice, t3[:, :, :, 0], t3[:, :, :, 1], op=ALU.max)


def _build_nc():
    nc = bacc.Bacc(
        "TRN2", target_bir_lowering=False, debug=False, num_devices=NCORES
    )

    xa = nc.declare_dram_parameter("xa", [128, R // 2], BF16, isOutput=False)
    wc = nc.declare_dram_parameter("wc", [128, HE], BF16, isOutput=False)
    # packed MLP weights [64, 141] f32: w1 rows 0:32 cols 0:64, w2 cols 64:128,
    # w3 cols 128:138, b1 col 138, b2 col 139, b3 col 140 (rows 0:10)
    wmlp = nc.declare_dram_parameter("wmlp", [64, 141], FP32, isOutput=False)
    out = nc.declare_dram_parameter("out", [NOUT, BPC], FP32, isOutput=True)

    with tile.TileContext(nc) as tc:
        with (
            tc.tile_pool(name="const", bufs=1) as const_pool,
            tc.tile_pool(name="xa", bufs=2) as xa_pool,
            tc.tile_pool(name="mb", bufs=2) as mb_pool,
            tc.tile_pool(name="yb", bufs=3) as yb_pool,
            tc.tile_pool(name="tree", bufs=2) as tree_pool,
            tc.tile_pool(name="mlp", bufs=1) as mlp_pool,
            tc.tile_pool(name="ypsum", bufs=3, space=bass.MemorySpace.PSUM) as ypsum_pool,
            tc.tile_pool(name="spsum", bufs=1, space=bass.MemorySpace.PSUM) as spsum_pool,
        ):
            # --- persistent tiles; first xa chunk + wc first (critical path) ---
            xa_tiles = []
            for c in range(4):
                t = xa_pool.tile([128, FCHUNK], BF16, tag="xa", name="xa_sb")
                xa_tiles.append(t)
            for piece in range(4):
                lo, hi = piece * 512, (piece + 1) * 512
                nc.sync.dma_start(xa_tiles[0][:, lo:hi], xa[:, lo:hi])
            wc_sb = const_pool.tile([128, HE], BF16)
            nc.sync.dma_start(wc_sb[:], wc[:])
            ones_sb = const_pool.tile([128, 1], BF16)
            nc.vector.memset(ones_sb[:], 1.0)

            # one PSUM bank shared by the S^T accumulator and the MLP matmuls
            sm_psum = spsum_pool.tile([64, 512], FP32)
            s_psum = sm_psum[0:H, 0:BPC]  # S^T accumulator

            # Main loop over 32 pairs (2 blocks each).
            mbs = [None] * (BPC)  # mb tile per batch
            pending_tree = None  # (ybf2, dst)
            pending_batch = []  # batches whose pooling is fully emitted

            def flush_tree():
                nonlocal pending_tree
                if pending_tree is not None:
                    ybf2, dst = pending_tree
                    _tree_max4(nc, tree_pool, ybf2, dst)
                    pending_tree = None

            def flush_batches():
                while pending_batch:
                    bi, mb_t = pending_batch.pop(0)
                    mbf = mb_t[:].rearrange("p a b -> p (a b)")
                    nc.vector.tensor_scalar_max(mbf, mbf, 0.0)
                    for j in range(8):
                        nc.tensor.matmul(
                            s_psum[:, bi : bi + 1],
                            mb_t[:, j, :],
                            ones_sb[:],
                            start=(j == 0),
                            stop=(j == 7),
                        )

            for pair in range(NBLK // 2):
                i = 2 * pair
                if i % (FCHUNK // 128) == 0 and i > 0:
                    c = i // (FCHUNK // 128)
                    nc.sync.dma_start(
                        xa_tiles[c][:], xa[:, c * FCHUNK : (c + 1) * FCHUNK]
                    )
                b_idx = i // 4
                slot = i % 4
                if slot == 0:
                    mbs[b_idx] = mb_pool.tile([128, 8, H], BF16, tag="mb", name="mb")
                mb = mbs[b_idx]

                path = PAIR_PATTERN[pair % len(PAIR_PATTERN)]
                ybf2 = None
                if path == "b":
                    ybf2 = yb_pool.tile([128, 4, H, E], BF16, tag="ybf", name="ybf")
                for q in range(2):  # the two blocks of the pair
                    blk = i + q
                    xa_sb = xa_tiles[blk // (FCHUNK // 128)]
                    f0 = (blk % (FCHUNK // 128)) * 128
                    y_ps = ypsum_pool.tile([128, 2 * HE], FP32, tag="y_ps", name="y_ps")
                    nc.tensor.matmul(
                        y_ps[:, 0:HE],
                        xa_sb[0:64, f0 : f0 + 128],
                        wc_sb[0:64, :],
                        start=True,
                        stop=True,
                    )
                    nc.tensor.matmul(
                        y_ps[:, HE : 2 * HE],
                        xa_sb[64:128, f0 : f0 + 128],
                        wc_sb[64:128, :],
                        start=True,
                        stop=True,
                    )
                    sl = slot + q
                    if path == "a":
                        nc.vector.tensor_reduce(
                            mb[:, 2 * sl : 2 * sl + 2, :],
                            y_ps[:].rearrange("p (t h e) -> p t h e", t=2, h=H, e=E),
                            axis=AX.X,
                            op=ALU.max,
                        )
                    else:
                        nc.scalar.activation(
                            ybf2[:, 2 * q : 2 * q + 2].rearrange(
                                "p a b c -> p (a b c)"
                            ),
                            y_ps[:],
                            ACT_F.Relu,
                        )
                if path == "b":
                    pending_tree = (ybf2, mb[:, 2 * slot : 2 * slot + 4, :])
                flush_tree()
                if slot + 1 == 3:
                    pending_batch.append((b_idx, mb))
                    flush_batches()
            flush_tree()
            flush_batches()

            # --- MLP tail (all transposed); weights arrive in one late DMA ---
            wmlp_sb = const_pool.tile([64, 141], FP32)
            nc.sync.dma_start(wmlp_sb[:], wmlp[:])
            w1_sb = wmlp_sb[0:H, 0:64]
            w2_sb = wmlp_sb[0:64, 64:128]
            w3_sb = wmlp_sb[0:64, 128 : 128 + NOUT]
            b1_sb = wmlp_sb[0:64, 138:139]
            b2_sb = wmlp_sb[0:64, 139:140]
            b3_sb = wmlp_sb[0:NOUT, 140:141]

            s_sb = mlp_pool.tile([H, BPC], FP32)
            nc.vector.tensor_copy(s_sb[:], s_psum[:])

            h1_ps = sm_psum[0:64, 64:80]
            nc.tensor.matmul(h1_ps, w1_sb, s_sb[:], start=True, stop=True)
            h1_sb = mlp_pool.tile([64, BPC], FP32)
            nc.scalar.activation(h1_sb[:], h1_ps, ACT_F.Relu, bias=b1_sb)

            h2_ps = sm_psum[0:64, 96:112]
            nc.tensor.matmul(h2_ps, w2_sb, h1_sb[:], start=True, stop=True)
            h2_sb = mlp_pool.tile([64, BPC], FP32)
            nc.scalar.activation(h2_sb[:], h2_ps, ACT_F.Relu, bias=b2_sb)

            o_ps = sm_psum[0:NOUT, 128:144]
            nc.tensor.matmul(o_ps, w3_sb, h2_sb[:], start=True, stop=True)
            o_sb = mlp_pool.tile([NOUT, BPC], FP32)
            nc.scalar.activation(o_sb[:], o_ps, ACT_F.Identity, bias=b3_sb)

            nc.sync.dma_start(out[:], o_sb[:])

    nc.compile()
    return nc


def _prep_shared(Wc, w1, b1, w2, b2, w3, b3):
    # reorder Wc columns: k = e*H + h  ->  k' = h*E + e
    Wc = np.asarray(Wc, dtype=np.float32)
    wc_r = np.ascontiguousarray(
        Wc.reshape(D, E, H).transpose(0, 2, 1).reshape(D, HE)
    )
    wc_stack = np.ascontiguousarray(
        np.concatenate([wc_r, wc_r], axis=0).astype(ml_dtypes.bfloat16)
    )
    wmlp = np.zeros((64, 141), np.float32)
    wmlp[0:H, 0:64] = np.asarray(w1, np.float32)
    wmlp[0:64, 64:128] = np.asarray(w2, np.float32)
    wmlp[0:64, 128 : 128 + NOUT] = np.asarray(w3, np.float32)
    wmlp[0:64, 138] = np.asarray(b1, np.float32)
    wmlp[0:64, 139] = np.asarray(b2, np.float32)
    wmlp[0:NOUT, 140] = np.asarray(b3, np.float32)
    return dict(wc=wc_stack, wmlp=wmlp)


def _pack_x(Xc):
    # Xc [BPC, P, D] -> A [128, R//2]: A[64*(r%2)+d, r//2] = Xc_flat[r, d]
    Xf = np.asarray(Xc, np.float32).reshape(R, D)
    A = Xf.reshape(R // 2, 2, D).transpose(1, 2, 0).reshape(128, R // 2)
    return np.ascontiguousarray(A.astype(ml_dtypes.bfloat16))


def run(X, Wc, w1, b1, w2, b2, w3, b3, trace=False):
    if "nc" not in _cache:
        _cache["nc"] = _build_nc()
    nc = _cache["nc"]

    shared = _prep_shared(Wc, w1, b1, w2, b2, w3, b3)
    in_maps = []
    for c in range(NCORES):
        m = dict(shared)
        m["xa"] = _pack_x(X[c * BPC : (c + 1) * BPC])
        in_maps.append(m)

    res = run_bass_kernel_spmd(
        nc, in_maps, core_ids=list(range(NCORES)), trace=trace
    )
    outs = [np.asarray(r["out"]).T for r in res.results]  # each [BPC, NOUT]
    full = np.concatenate(outs, axis=0).astype(np.float32)
    return full, res


def kernel(X, Wc, w1, b1, w2, b2, w3, b3):
    full, _ = run(X, Wc, w1, b1, w2, b2, w3, b3, trace=False)
    return full


# revision 25
# speedup vs baseline: 1.7536x; 1.0613x over previous
"""ApproxRepSet kernel for 8 TRN2 NeuronCores.

reference:
  t = relu(X @ Wc)            # [B, P, H*E], k = e*H + h
  t = max over e              # [B, P, H]
  t = sum over p              # [B, H]
  t = relu(t @ w1 + b1); t = relu(t @ w2 + b2); out = t @ w3 + b3

Sharding: data-parallel over batch, 16 batches per core. Weights replicated.

Per-core layout tricks (all host-side, zero on-device transposes):
  - X shard [16*1024, 64] is packed as A[128, 8192]: partition 64*(r%2)+d,
    free r//2.  A 256-row block i lives at free cols [128i, 128i+128): even
    rows on partitions 0:64, odd rows on partitions 64:128.  Each half is
    directly the matmul stationary lhsT [K=64, M=128]; the two halves run
    concurrently via PE row tiling (tile_position (0,0) and (64,0)).
  - X/Wc cast to bf16 on host: halves DMA bytes and avoids the fp32
    double-pass through the PE (fp32 matmul lowers to 2 instructions).
  - Wc columns reordered k' = h*16 + e, so max over e is an innermost
    free-dim window reduce.  Wc stacked twice on partitions for row tiling.
  - The max-over-e pooling is the throughput wall (every Y element must be
    read out of PSUM by DVE or ACT at ~1 elem/lane/cycle; DMA and GPSIMD
    have no PSUM route, and GPSIMD TensorTensor fails walrus codegen).
    Block pairs are split across two pipelines so DVE and ACT pool in
    parallel:
      path a: DVE tensor_reduce(max) straight from PSUM    (~1.17us/blk DVE)
      path b: ACT relu-cast PSUM->SBUF bf16, then one batched DVE TT-max
              tree per pair                        (~1.07 ACT + ~0.67 DVE /blk)
  - Sum over p: ones-vector matmuls (lhsT = pooled tile [128, 32],
    rhs = ones [128,1]) accumulating S^T [32, 16] in PSUM.
  - MLP stays transposed end-to-end: h1^T = w1^T @ S^T etc., so w1/w2/w3 are
    used in natural layout as lhsT.
"""

import sys

import numpy as np

sys.path.insert(0, "/opt/trn_rl_repo")

import ml_dtypes
import concourse.bass as bass
import concourse.mybir as mybir
import concourse.tile as tile
from concourse import bacc
from concourse.bass_utils import run_bass_kernel_spmd

B, P, D = 128, 1024, 64
H, E = 32, 16
HE = H * E  # 512
NOUT = 10
NCORES = 8
BPC = B // NCORES  # 16 batches per core
R = BPC * P  # 16384 rows per core
NBLK = R // 256  # 64 blocks of 256 rows
FCHUNK = 2048  # free-dim cols per DMA chunk (= 16 blocks)

FP32 = mybir.dt.float32
BF16 = mybir.dt.bfloat16
AX = mybir.AxisListType
ALU = mybir.AluOpType
ACT_F = mybir.ActivationFunctionType

# Per-batch block roles: "a" = DVE tensor_reduce straight from PSUM,
# "b" = ACT relu-cast to bf16 + DVE TT-max tree.  [a,b,b,b] batches keep both
# engines fed concurrently; two all-b batches balance total DVE vs ACT load.
ALLB_BATCHES = {7, 15}

_cache = {}


def _tree_max4(nc, pool, src, dst_slice):
    """4-level DVE TT-max tree over innermost e=16 of src [128, 4, H, 16] bf16
    (two blocks), writing [128, 4, H] into dst_slice."""
    t1 = pool.tile([128, 4, H, 8], BF16, tag="t1", name="t1")
    nc.vector.tensor_tensor(t1[:], src[:, :, :, 0:8], src[:, :, :, 8:16], op=ALU.max)
    t2 = pool.tile([128, 4, H, 4], BF16, tag="t2", name="t2")
    nc.vector.tensor_tensor(t2[:], t1[:, :, :, 0:4], t1[:, :, :, 4:8], op=ALU.max)
    t3 = pool.tile([128, 4, H, 2], BF16, tag="t3", name="t3")
    nc.vector.tensor_tensor(t3[:], t2[:, :, :, 0:2], t2[:, :, :, 2:4], op=ALU.max)
    nc.vector.tensor_tensor(dst_slice, t3[:, :, :, 0], t3[:, :, :, 1], op=ALU.max)


def _build_nc():
    nc = bacc.Bacc(
        "TRN2", target_bir_lowering=False, debug=False, num_devices=NCORES
    )

    xa = nc.declare_dram_parameter("xa", [128, R // 2], BF16, isOutput=False)
    wc = nc.declare_dram_parameter("wc", [128, HE], BF16, isOutput=False)
    # packed MLP weights [64, 141] f32: w1 rows 0:32 cols 0:64, w2 cols 64:128,
    # w3 cols 128:138, b1 col 138, b2 col 139, b3 col 140 (rows 0:10)
    wmlp = nc.declare_dram_parameter("wmlp", [64, 141], FP32, isOutput=False)
    out = nc.declare_dram_parameter("out", [NOUT, BPC], FP32, isOutput=True)

    with tile.TileContext(nc) as tc:
        with (
            tc.tile_pool(name="const", bufs=1) as const_pool,
            tc.tile_pool(name="xa", bufs=2) as xa_pool,
            tc.tile_pool(name="mb", bufs=2) as mb_pool,
            tc.tile_pool(name="yb", bufs=3) as yb_pool,
            tc.tile_pool(name="tree", bufs=2) as tree_pool,
            tc.tile_pool(name="mlp", bufs=1) as mlp_pool,
            tc.tile_pool(name="ypsum", bufs=3, space=bass.MemorySpace.PSUM) as ypsum_pool,
            tc.tile_pool(name="spsum", bufs=1, space=bass.MemorySpace.PSUM) as spsum_pool,
        ):
            # --- persistent tiles; first xa chunk + wc first (critical path) ---
            xa_tiles = []
            for c in range(4):
                t = xa_pool.tile([128, FCHUNK], BF16, tag="xa", name="xa_sb")
                xa_tiles.append(t)
            for piece in range(4):
                lo, hi = piece * 512, (piece + 1) * 512
                nc.sync.dma_start(xa_tiles[0][:, lo:hi], xa[:, lo:hi])
            wc_sb = const_pool.tile([128, HE], BF16)
            nc.gpsimd.dma_start(wc_sb[:], wc[:])
            ones_sb = const_pool.tile([128, 1], BF16)
            nc.vector.memset(ones_sb[:], 1.0)

            # one PSUM bank shared by the S^T accumulator and the MLP matmuls
            sm_psum = spsum_pool.tile([64, 512], FP32)
            s_psum = sm_psum[0:H, 0:BPC]  # S^T accumulator

            # Main loop: 16 batches x 4 blocks, block-granular a/b roles
            def tree2(src, dst_slice):
                # 4-level TT-max tree for ONE block: src [128, 2, H, 16] bf16
                u1 = tree_pool.tile([128, 2, H, 8], BF16, tag="u1", name="u1")
                nc.vector.tensor_tensor(
                    u1[:], src[:, :, :, 0:8], src[:, :, :, 8:16], op=ALU.max
                )
                u2 = tree_pool.tile([128, 2, H, 4], BF16, tag="u2", name="u2")
                nc.vector.tensor_tensor(
                    u2[:], u1[:, :, :, 0:4], u1[:, :, :, 4:8], op=ALU.max
                )
                u3 = tree_pool.tile([128, 2, H, 2], BF16, tag="u3", name="u3")
                nc.vector.tensor_tensor(
                    u3[:], u2[:, :, :, 0:2], u2[:, :, :, 2:4], op=ALU.max
                )
                nc.vector.tensor_tensor(
                    dst_slice, u3[:, :, :, 0], u3[:, :, :, 1], op=ALU.max
                )

            def do_block(blk, role, mb, sl, ybf_slot):
                xa_sb = xa_tiles[blk // (FCHUNK // 128)]
                f0 = (blk % (FCHUNK // 128)) * 128
                y_ps = ypsum_pool.tile([128, 2 * HE], FP32, tag="y_ps", name="y_ps")
                nc.tensor.matmul(
                    y_ps[:, 0:HE],
                    xa_sb[0:64, f0 : f0 + 128],
                    wc_sb[0:64, :],
                    start=True,
                    stop=True,
                )
                nc.tensor.matmul(
                    y_ps[:, HE : 2 * HE],
                    xa_sb[64:128, f0 : f0 + 128],
                    wc_sb[64:128, :],
                    start=True,
                    stop=True,
                )
                if role == "a":
                    nc.vector.tensor_reduce(
                        mb[:, 2 * sl : 2 * sl + 2, :],
                        y_ps[:].rearrange("p (t h e) -> p t h e", t=2, h=H, e=E),
                        axis=AX.X,
                        op=ALU.max,
                    )
                else:
                    nc.scalar.activation(
                        ybf_slot.rearrange("p a b c -> p (a b c)"),
                        y_ps[:],
                        ACT_F.Relu,
                    )

            for b_idx in range(BPC):
                i0 = 4 * b_idx
                if i0 % (FCHUNK // 128) == 0 and i0 > 0:
                    c = i0 // (FCHUNK // 128)
                    nc.sync.dma_start(
                        xa_tiles[c][:], xa[:, c * FCHUNK : (c + 1) * FCHUNK]
                    )
                mb = mb_pool.tile([128, 8, H], BF16, tag="mb", name="mb")

                if b_idx in ALLB_BATCHES:  # four b-blocks, two pair-trees
                    yb_a = yb_pool.tile([128, 4, H, E], BF16, tag="ybf", name="ybf")
                    yb_b = yb_pool.tile([128, 4, H, E], BF16, tag="ybf", name="ybf")
                    do_block(i0 + 0, "b", mb, 0, yb_a[:, 0:2])
                    do_block(i0 + 1, "b", mb, 1, yb_a[:, 2:4])
                    _tree_max4(nc, tree_pool, yb_a, mb[:, 0:4, :])
                    do_block(i0 + 2, "b", mb, 2, yb_b[:, 0:2])
                    do_block(i0 + 3, "b", mb, 3, yb_b[:, 2:4])
                    _tree_max4(nc, tree_pool, yb_b, mb[:, 4:8, :])
                else:  # a, b, b, b
                    yb_a = yb_pool.tile([128, 4, H, E], BF16, tag="ybf", name="ybf")
                    yb_c = yb_pool.tile([128, 2, H, E], BF16, tag="ybf1", name="ybf1")
                    do_block(i0 + 0, "a", mb, 0, None)
                    do_block(i0 + 1, "b", mb, 1, yb_a[:, 0:2])
                    do_block(i0 + 2, "b", mb, 2, yb_a[:, 2:4])
                    _tree_max4(nc, tree_pool, yb_a, mb[:, 2:6, :])
                    do_block(i0 + 3, "b", mb, 3, yb_c[:])
                    tree2(yb_c, mb[:, 6:8, :])

                mbf = mb[:].rearrange("p a b -> p (a b)")
                nc.vector.tensor_scalar_max(mbf, mbf, 0.0)
                for j in range(8):
                    nc.tensor.matmul(
                        s_psum[:, b_idx : b_idx + 1],
                        mb[:, j, :],
                        ones_sb[:],
                        start=(j == 0),
                        stop=(j == 7),
                    )

            # --- MLP tail (all transposed); weights arrive in one late DMA ---
            wmlp_sb = const_pool.tile([64, 141], FP32)
            nc.gpsimd.dma_start(wmlp_sb[:], wmlp[:])
            w1_sb = wmlp_sb[0:H, 0:64]
            w2_sb = wmlp_sb[0:64, 64:128]
            w3_sb = wmlp_sb[0:64, 128 : 128 + NOUT]
            b1_sb = wmlp_sb[0:64, 138:139]
            b2_sb = wmlp_sb[0:64, 139:140]
            b3_sb = wmlp_sb[0:NOUT, 140:141]

            s_sb = mlp_pool.tile([H, BPC], FP32)
            nc.vector.tensor_copy(s_sb[:], s_psum[:])

            h1_ps = sm_psum[0:64, 64:80]
            nc.tensor.matmul(h1_ps, w1_sb, s_sb[:], start=True, stop=True)
            h1_sb = mlp_pool.tile([64, BPC], FP32)
            nc.scalar.activation(h1_sb[:], h1_ps, ACT_F.Relu, bias=b1_sb)

            h2_ps = sm_psum[0:64, 96:112]
            nc.tensor.matmul(h2_ps, w2_sb, h1_sb[:], start=True, stop=True)
            h2_sb = mlp_pool.tile([64, BPC], FP32)
            nc.scalar.activation(h2_sb[:], h2_ps, ACT_F.Relu, bias=b2_sb)

            o_ps = sm_psum[0:NOUT, 128:144]
            nc.tensor.matmul(o_ps, w3_sb, h2_sb[:], start=True, stop=True)
            o_sb = mlp_pool.tile([NOUT, BPC], FP32)
            nc.scalar.activation(o_sb[:], o_ps, ACT_F.Identity, bias=b3_sb)

            nc.sync.dma_start(out[:], o_sb[:])

    nc.compile()
    return nc


def _prep_shared(Wc, w1, b1, w2, b2, w3, b3):
    # reorder Wc columns: k = e*H + h  ->  k' = h*E + e
    Wc = np.asarray(Wc, dtype=np.float32)
    wc_r = np.ascontiguousarray(
        Wc.reshape(D, E, H).transpose(0, 2, 1).reshape(D, HE)
    )
    wc_stack = np.ascontiguousarray(
        np.concatenate([wc_r, wc_r], axis=0).astype(ml_dtypes.bfloat16)
    )
    wmlp = np.zeros((64, 141), np.float32)
    wmlp[0:H, 0:64] = np.asarray(w1, np.float32)
    wmlp[0:64, 64:128] = np.asarray(w2, np.float32)
    wmlp[0:64, 128 : 128 + NOUT] = np.asarray(w3, np.float32)
    wmlp[0:64, 138] = np.asarray(b1, np.float32)
    wmlp[0:64, 139] = np.asarray(b2, np.float32)
    wmlp[0:NOUT, 140] = np.asarray(b3, np.float32)
    return dict(wc=wc_stack, wmlp=wmlp)


def _pack_x(Xc):
    # Xc [BPC, P, D] -> A [128, R//2]: A[64*(r%2)+d, r//2] = Xc_flat[r, d]
    Xf = np.asarray(Xc, np.float32).reshape(R, D)
    A = Xf.reshape(R // 2, 2, D).transpose(1, 2, 0).reshape(128, R // 2)
    return np.ascontiguousarray(A.astype(ml_dtypes.bfloat16))


def run(X, Wc, w1, b1, w2, b2, w3, b3, trace=False):
    if "nc" not in _cache:
        _cache["nc"] = _build_nc()
    nc = _cache["nc"]

    shared = _prep_shared(Wc, w1, b1, w2, b2, w3, b3)
    in_maps = []
    for c in range(NCORES):
        m = dict(shared)
        m["xa"] = _pack_x(X[c * BPC : (c + 1) * BPC])
        in_maps.append(m)

    res = run_bass_kernel_spmd(
        nc, in_maps, core_ids=list(range(NCORES)), trace=trace
    )
    outs = [np.asarray(r["out"]).T for r in res.results]  # each [BPC, NOUT]
    full = np.concatenate(outs, axis=0).astype(np.float32)
    return full, res


def kernel(X, Wc, w1, b1, w2, b2, w3, b3):
    full, _ = run(X, Wc, w1, b1, w2, b2, w3, b3, trace=False)
    return full
